# revision 21
# baseline (speedup 1.0000x reference)
"""Trainium2 Bass kernel for nn_BertSelfOutputPAL.

Data-parallel over batch: 8 batch elements -> 8 NeuronCores, no collectives.
Per core (batch element b), with S=2048, H=1024, P=256, T=4:
  h   = hs @ W + b                       (dense)
  low_t = h @ W1[t] + b1[t]              (PAL down-proj, T branches)
  ts_t  = low_t @ W2[t] + b2[t]          (PAL up-proj)
  tw  = softmax(h @ enc_W + mask)        (token gate over S)
  tv  = tw @ h
  td  = softmax(tv @ sel_W.T + sel_b)    (task gate over T)
  x   = h + input + sum_t td[t] * ts_t
  out = LayerNorm(x) * g + beta

On-chip layout: activations are kept feature-major ("hT": [H partitions, S
free]) so natural-layout weights slice directly as matmul stationary
operands. The residual/LayerNorm stage runs in natural layout [S, H]; h is
transposed back via PE-transpose matmuls accumulated directly into the
stage-2 PSUM tile. Matmuls run as float32r (full-rate fp32 data).
"""

import os
import numpy as np
from contextlib import ExitStack

import concourse.bacc as bacc
import concourse.bass as bass
import concourse.mybir as mybir
import concourse.tile as tile
from concourse.bass_utils import run_bass_kernel_spmd
from concourse.masks import make_identity

FP = mybir.dt.float32
FR = mybir.dt.float32r
AF = mybir.ActivationFunctionType
ALU = mybir.AluOpType
AX = mybir.AxisListType
EPS = 1e-12

B, S_FULL, H, P, T = 8, 2048, 1024, 256, 4
KT = H // 128      # 8 h-tiles
PT = P // 128      # 2 p-tiles
N_CORES = 8


def fr(ap):
    return ap.bitcast(FR)


def build_nc(S=S_FULL, zb2=False, zmask=False, zg=False, zb=False):
    SC = S // 512            # 512-wide s-chunks
    NST = S // 128           # 128-row s-tiles
    nc = bacc.Bacc("TRN2", target_bir_lowering=False, debug=False)

    # ---- DRAM I/O (per-core) ----
    hs = nc.dram_tensor("hs", [S, H], FR, kind="ExternalInput").ap()
    inp = nc.dram_tensor("inp", [S, H], FP, kind="ExternalInput").ap()
    mask = nc.dram_tensor("mask", [1, S], FP, kind="ExternalInput").ap()
    Wd = nc.dram_tensor("Wd", [H, H], FR, kind="ExternalInput").ap()
    dbias_d = nc.dram_tensor("dbias", [128, KT], FP, kind="ExternalInput").ap()
    W1d = nc.dram_tensor("W1", [T, H, P], FR, kind="ExternalInput").ap()
    b1_d = nc.dram_tensor("b1", [128, PT, T], FP, kind="ExternalInput").ap()
    W2d = nc.dram_tensor("W2", [T, P, H], FR, kind="ExternalInput").ap()
    b2_d = nc.dram_tensor("b2", [T, H], FR, kind="ExternalInput").ap()
    encw_d = nc.dram_tensor("encw", [128, KT], FR, kind="ExternalInput").ap()
    selw_d = nc.dram_tensor("selw", [128, KT, T], FR, kind="ExternalInput").ap()
    selb_d = nc.dram_tensor("selb", [1, T], FP, kind="ExternalInput").ap()
    lng_d = nc.dram_tensor("lng", [1, H], FP, kind="ExternalInput").ap()
    lnb_d = nc.dram_tensor("lnb", [1, H], FP, kind="ExternalInput").ap()
    outp = nc.dram_tensor("out", [S, H], FP, kind="ExternalOutput").ap()

    with tile.TileContext(nc) as tc, ExitStack() as ctx:
        # ---------- persistent pools ----------
        persist = ctx.enter_context(tc.tile_pool(name="persist", bufs=1))
        htp = ctx.enter_context(tc.tile_pool(name="htp", bufs=1))

        ident = persist.tile([128, 128], FP, tag="ident")
        make_identity(nc, ident[:])
        ones1 = persist.tile([1, 128], FR, tag="ones1")
        nc.gpsimd.memset(ones1[:], 1.0)

        dbias = persist.tile([128, KT], FP, tag="dbias")
        nc.sync.dma_start(dbias[:], dbias_d)
        b1s = persist.tile([128, PT, T], FP, tag="b1s")
        nc.sync.dma_start(b1s[:], b1_d)
        encw = persist.tile([128, KT], FR, tag="encw")
        nc.sync.dma_start(encw[:], encw_d)
        selw = persist.tile([128, KT, T], FR, tag="selw")
        nc.sync.dma_start(selw[:], selw_d)
        selb = persist.tile([1, T], FP, tag="selb")
        nc.sync.dma_start(selb[:], selb_d)
        b2n = persist.tile([T, H], FP, tag="b2n")
        nc.sync.dma_start(b2n[:], b2_d)
        lngr = persist.tile([1, H], FP, tag="lngr")
        nc.sync.dma_start(lngr[:], lng_d)
        lnbr = persist.tile([1, H], FP, tag="lnbr")
        nc.sync.dma_start(lnbr[:], lnb_d)
        lngb = persist.tile([128, H], FP, tag="lngb")
        nc.gpsimd.partition_broadcast(lngb[:], lngr[:])
        lnbb = persist.tile([128, H], FP, tag="lnbb")
        nc.gpsimd.partition_broadcast(lnbb[:], lnbr[:])

        # logits row; reused in place as exp/tw
        lgrow = persist.tile([1, S], FP, tag="lgrow")
        tvcols = persist.tile([128, KT], FP, tag="tvcols")
        td_row = persist.tile([1, T], FP, tag="td_row")
        tdcol = persist.tile([T, 1], FP, tag="tdcol")
        b2c = persist.tile([1, H], FP, tag="b2c")
        tdb = persist.tile([128, T], FP, tag="tdb")

        # hT: feature-major h, [128, S] x KT tiles (persists all passes)
        hT = [htp.tile([128, S], FP, tag=f"ht{k}") for k in range(KT)]

        # ================= pass A: transpose X, dense, logits =================
        with tc.tile_pool(name="pA", bufs=1) as pa, \
             tc.tile_pool(name="pA_xn", bufs=6) as pxn, \
             tc.tile_pool(name="pA_xt", bufs=2 * KT) as pxt, \
             tc.tile_pool(name="pA_ps_t", bufs=3, space="PSUM") as tps, \
             tc.tile_pool(name="pA_ps_d", bufs=3, space="PSUM") as dps, \
             tc.tile_pool(name="pA_ps_l", bufs=1, space="PSUM") as lps:
            W_sb = [pa.tile([128, H], FR, tag=f"wd{k}") for k in range(KT)]
            for k in range(KT):
                nc.sync.dma_start(W_sb[k][:], Wd[k * 128:(k + 1) * 128, :])
            mrow = pa.tile([1, S], FP, tag="mrow")
            nc.sync.dma_start(mrow[:], mask)

            for c in range(SC):
                # load X natural rows for this chunk
                xn = []
                for st in range(4):
                    t_ = pxn.tile([128, H], FP, tag="xn")
                    nc.sync.dma_start(t_[:], hs[(c * 4 + st) * 128:(c * 4 + st + 1) * 128, :])
                    xn.append(t_)
                # transpose -> XT chunk tiles [128(h), 512(s)]
                xtc = []
                for kt in range(KT):
                    ps = tps.tile([128, 512], FP, tag="tps")
                    for st in range(4):
                        nc.tensor.transpose(
                            fr(ps[:, st * 128:(st + 1) * 128]),
                            xn[st][:, kt * 128:(kt + 1) * 128],
                            identr[:],
                        )
                    xt_t = pxt.tile([128, 512], FP, tag="xt")
                    nc.scalar.copy(fr(xt_t[:]), ps[:])
                    xtc.append(xt_t)
                # dense: hT[mt][:, chunk] = sum_k W[k,mt].T @ XT[k] (+bias)
                for mt in range(KT):
                    ps = dps.tile([128, 512], FP, tag="dps")
                    for kt in range(KT):
                        nc.tensor.matmul(
                            ps[:],
                            fr(W_sb[kt][:, mt * 128:(mt + 1) * 128]),
                            fr(xtc[kt][:]),
                            start=(kt == 0),
                            stop=(kt == KT - 1),
                        )
                    nc.scalar.activation(
                        fr(hT[mt][:, c * 512:(c + 1) * 512]), ps[:], AF.Identity,
                        bias=dbias[:, mt:mt + 1], scale=1.0,
                    )
                # logits for this chunk (uses hT)
                ps = lps.tile([1, 512], FP, tag="lps")
                for kt in range(KT):
                    nc.tensor.matmul(
                        ps[:],
                        fr(encw[:, kt:kt + 1]),
                        fr(hT[kt][:, c * 512:(c + 1) * 512]),
                        start=(kt == 0),
                        stop=(kt == KT - 1),
                    )
                # logits + mask -> lgrow chunk
                if zmask:
                    nc.scalar.copy(lgrow[:, c * 512:(c + 1) * 512], ps[:])
                else:
                    nc.vector.tensor_add(
                        lgrow[:, c * 512:(c + 1) * 512], ps[:],
                        mrow[:, c * 512:(c + 1) * 512],
                    )

        # ================= W1/W2 loads (region freed by pass A) ===============
        w12 = ctx.enter_context(tc.tile_pool(name="w12", bufs=1))
        W1_sb = {}
        for t in range(T):
            for kt in range(KT):
                w1t = w12.tile([128, P], FR, tag=f"w1_{t}_{kt}")
                nc.scalar.dma_start(w1t[:], W1d[t, kt * 128:(kt + 1) * 128, :])
                W1_sb[(t, kt)] = w1t
        W2_sb = {}
        for t in range(T):
            for pt in range(PT):
                w2t = w12.tile([128, H], FR, tag=f"w2_{t}_{pt}")
                nc.scalar.dma_start(w2t[:], W2d[t, pt * 128:(pt + 1) * 128, :])
                W2_sb[(t, pt)] = w2t

        lowp = ctx.enter_context(tc.tile_pool(name="lowp", bufs=2 * T * PT))
        lowps = ctx.enter_context(
            tc.tile_pool(name="lowps", bufs=3 if zb2 else 2, space="PSUM"))
        xps = ctx.enter_context(tc.tile_pool(name="xps", bufs=4, space="PSUM"))
        bps = ctx.enter_context(
            tc.tile_pool(name="bps", bufs=1 if zb2 else 2, space="PSUM"))

        # ================= pass B: softmax, tv, td, scale W2 ==================
        with tc.tile_pool(name="pB", bufs=1) as pb:
            mx = pb.tile([1, 1], FP, tag="mx")
            nc.vector.reduce_max(mx[:], lgrow[:], axis=AX.X)
            negmx = pb.tile([1, 1], FP, tag="negmx")
            nc.vector.tensor_scalar(negmx[:], mx[:], -1.0, None, op0=ALU.mult)
            zsum = pb.tile([1, 1], FP, tag="zsum")
            # lgrow <- exp(lgrow - mx), zsum = sum(exp)
            nc.scalar.activation(lgrow[:], lgrow[:], AF.Exp,
                                 bias=negmx[:], scale=1.0, accum_out=zsum[:])
            rz = pb.tile([1, 1], FP, tag="rz")
            nc.vector.reciprocal(rz[:], zsum[:])
            # lgrow <- tw = exp * (1/Z)
            nc.vector.tensor_scalar(lgrow[:], lgrow[:], rz[:], None, op0=ALU.mult)
            twb = pb.tile([128, S], FP, tag="twb")
            nc.gpsimd.partition_broadcast(twb[:], lgrow[:])
            # tv[h] = sum_s hT[h,s]*tw[s]  (DVE fused mul+reduce, halves)
            scr = pb.tile([128, S // 2], FP, tag="scr")
            tva = pb.tile([128, KT], FP, tag="tva")
            tvb_ = pb.tile([128, KT], FP, tag="tvb")
            HF = S // 2
            for kt in range(KT):
                nc.vector.scalar_tensor_tensor(
                    scr[:], hT[kt][:, :HF], 1.0, twb[:, :HF],
                    op0=ALU.mult, op1=ALU.mult, accum_out=tva[:, kt:kt + 1])
                nc.vector.scalar_tensor_tensor(
                    scr[:], hT[kt][:, HF:], 1.0, twb[:, HF:],
                    op0=ALU.mult, op1=ALU.mult, accum_out=tvb_[:, kt:kt + 1])
            nc.vector.tensor_add(fr(tvcols[:]), tva[:], tvb_[:])
            # td logits = tv @ selW.T + selb  -> [1, T]
            ps = bps.tile([1, T], FP, tag="bmisc")
            for kt in range(KT):
                nc.tensor.matmul(ps[:], fr(tvcols[:, kt:kt + 1]), fr(selw[:, kt, :]),
                                 start=(kt == 0), stop=(kt == KT - 1))
            tdl = pb.tile([1, T], FP, tag="tdl")
            nc.vector.tensor_add(tdl[:], ps[:], selb[:])
            # softmax over T
            mx2 = pb.tile([1, 1], FP, tag="mx2")
            nc.vector.reduce_max(mx2[:], tdl[:], axis=AX.X)
            negmx2 = pb.tile([1, 1], FP, tag="negmx2")
            nc.vector.tensor_scalar(negmx2[:], mx2[:], -1.0, None, op0=ALU.mult)
            z2 = pb.tile([1, 1], FP, tag="z2")
            nc.scalar.activation(tdl[:], tdl[:], AF.Exp, bias=negmx2[:], scale=1.0,
                                 accum_out=z2[:])
            rz2 = pb.tile([1, 1], FP, tag="rz2")
            nc.vector.reciprocal(rz2[:], z2[:])
            nc.vector.tensor_scalar(fr(td_row[:]), tdl[:], rz2[:], None, op0=ALU.mult)
            # td as column [T,1] via K=1 matmul: td_row.T @ [[1]]
            ps2 = bps.tile([T, 1], FP, tag="bmisc")
            nc.tensor.matmul(ps2[:], fr(td_row[:]), fr(ones1[:, :1]),
                             start=True, stop=True)
            nc.scalar.copy(fr(tdcol[:]), ps2[:])
            # b2c row = td @ b2  [1, H]
            for hc in range(2):
                ps3 = bps.tile([1, 512], FP, tag="bmisc")
                nc.tensor.matmul(ps3[:], fr(tdcol[:]),
                                 fr(b2n[:, hc * 512:(hc + 1) * 512]),
                                 start=True, stop=True)
                nc.scalar.copy(fr(b2c[:, hc * 512:(hc + 1) * 512]), ps3[:])
            # td broadcast [128, T]; scale W2 in place
            nc.gpsimd.partition_broadcast(tdb[:], td_row[:])
            for t in range(T):
                for pt in range(PT):
                    w2t = W2_sb[(t, pt)]
                    nc.vector.tensor_scalar(fr(w2t[:]), w2t[:], tdb[:, t:t + 1], None,
                                            op0=ALU.mult)

        # ================= pass C: low, stage2(+transpose+bias), LN ===========
        st3 = ctx.enter_context(tc.tile_pool(name="st3", bufs=1))
        xt_pool = ctx.enter_context(tc.tile_pool(name="xt3", bufs=2))
        in_pool = ctx.enter_context(tc.tile_pool(name="in3", bufs=2))
        stats = ctx.enter_context(tc.tile_pool(name="stats", bufs=4))

        low_tiles = {}

        def emit_low(c):
            for t in range(T):
                for pt in range(PT):
                    ps = lowps.tile([128, 512], FP, tag="lowps")
                    for kt in range(KT):
                        nc.tensor.matmul(
                            ps[:],
                            fr(W1_sb[(t, kt)][:, pt * 128:(pt + 1) * 128]),
                            fr(hT[kt][:, c * 512:(c + 1) * 512]),
                            start=(kt == 0), stop=(kt == KT - 1),
                        )
                    lt = lowp.tile([128, 512], FP, tag="low")
                    nc.scalar.activation(fr(lt[:]), ps[:], AF.Identity,
                                         bias=b1s[:, pt:pt + 1, t:t + 1], scale=1.0)
                    low_tiles[(c, t, pt)] = lt

        def emit_stage23(c):
            for st in range(4):
                s_abs = c * 4 + st
                pss = []
                for hc in range(2):
                    ps = xps.tile([128, 512], FP, tag="xps")
                    k = 0
                    for t in range(T):
                        for pt in range(PT):
                            nc.tensor.matmul(
                                ps[:],
                                fr(low_tiles[(c, t, pt)][:, st * 128:(st + 1) * 128]),
                                fr(W2_sb[(t, pt)][:, hc * 512:(hc + 1) * 512]),
                                start=(k == 0), stop=False,
                            )
                            k += 1
                    # accumulate h_nat via PE transpose of hT into same psum
                    for j in range(4):
                        kt = hc * 4 + j
                        nc.tensor.matmul(
                            fr(ps[:, j * 128:(j + 1) * 128]),
                            fr(hT[kt][:, s_abs * 128:(s_abs + 1) * 128]),
                            identr[:],
                            is_transpose=True, start=False, stop=(zb2 and j == 3),
                        )
                    if not zb2:
                        # + b2c broadcast row (K=1 rank-1 update), ends group
                        nc.tensor.matmul(
                            ps[:], ones1[:], fr(b2c[:, hc * 512:(hc + 1) * 512]),
                            start=False, stop=True,
                        )
                    pss.append(ps)
                # ---- stage 3: x = psum + input; LayerNorm ----
                it = in_pool.tile([128, H], FP, tag="inp")
                nc.sync.dma_start(it[:], inp[s_abs * 128:(s_abs + 1) * 128, :])
                xt_ = xt_pool.tile([128, H], FP, tag="x")
                s0 = stats.tile([128, 1], FP, tag="s0")
                s1 = stats.tile([128, 1], FP, tag="s1")
                for hc, sacc in ((0, s0), (1, s1)):
                    nc.vector.scalar_tensor_tensor(
                        xt_[:, hc * 512:(hc + 1) * 512], pss[hc][:], 0.0,
                        it[:, hc * 512:(hc + 1) * 512],
                        op0=ALU.add, op1=ALU.add, accum_out=sacc[:])
                sq = sq_pool.tile([128, H], FP, tag="sq")
                ssq = stats.tile([128, 1], FP, tag="ssq")
                nc.scalar.activation(sq[:], xt_[:], AF.Square, bias=zerot[:], accum_out=ssq[:])
                ssum = stats.tile([128, 1], FP, tag="ssum")
                nc.vector.tensor_add(ssum[:], s0[:], s1[:])
                mu = stats.tile([128, 1], FP, tag="mu")
                nc.vector.tensor_scalar(mu[:], ssum[:], 1.0 / H, None, op0=ALU.mult)
                musq = stats.tile([128, 1], FP, tag="musq")
                nc.vector.tensor_mul(musq[:], mu[:], mu[:])
                var = stats.tile([128, 1], FP, tag="var")
                nc.vector.tensor_scalar(var[:], ssq[:], 1.0 / H, musq[:],
                                        op0=ALU.mult, op1=ALU.subtract)
                sd = stats.tile([128, 1], FP, tag="sd")
                nc.scalar.activation(sd[:], var[:], AF.Sqrt, bias=epst[:], scale=1.0)
                isd = stats.tile([128, 1], FP, tag="isd")
                nc.vector.reciprocal(isd[:], sd[:])
                # x <- (x - mu) * isd   (2x-mode tensor_scalar)
                nc.vector.tensor_scalar(xt_[:], xt_[:], mu[:], isd[:],
                                        op0=ALU.subtract, op1=ALU.mult)
                if not zg:
                    # x <- x * g
                    nc.vector.scalar_tensor_tensor(xt_[:], xt_[:], 1.0, lngb[:],
                                                   op0=ALU.mult, op1=ALU.mult)
                if not zb:
                    # x <- x + beta (gpsimd), then store
                    nc.gpsimd.tensor_add(xt_[:], xt_[:], lnbb[:])
                nc.sync.dma_start(outp[s_abs * 128:(s_abs + 1) * 128, :], xt_[:])

        # software pipeline: keep 2 chunks of low tiles in flight
        emit_low(0)
        for c in range(1, SC):
            emit_low(c)
            emit_stage23(c - 1)
        emit_stage23(SC - 1)

    nc.finalize()
    return nc


_CACHE = {}


def _get_nc(S=S_FULL, zb2=False, zmask=False, zg=False, zb=False):
    key = (S, zb2, zmask, zg, zb)
    if key not in _CACHE:
        _CACHE[key] = build_nc(S, zb2=zb2, zmask=zmask, zg=zg, zb=zb)
    return _CACHE[key]


def _flags(inputs):
    f32 = lambda x: np.asarray(x, dtype=np.float32)
    return dict(
        zb2=not np.any(f32(inputs["pal_b2"])),
        zmask=not np.any(f32(inputs["attention_mask"])),
        zg=bool(np.all(f32(inputs["ln_g"]) == 1.0)),
        zb=not np.any(f32(inputs["ln_b"])),
    )


def _in_maps(inputs, S=S_FULL):
    f32 = lambda x: np.ascontiguousarray(np.asarray(x), dtype=np.float32)
    hs = f32(inputs["hidden_states"])
    inp = f32(inputs["input_tensor"])
    msk = f32(inputs["attention_mask"]).reshape(B, S)
    Wd = f32(inputs["dense_W"])
    dbias = f32(inputs["dense_b"]).reshape(KT, 128).T.copy()
    W1 = f32(inputs["pal_W1"])
    b1 = f32(inputs["pal_b1"]).reshape(T, PT, 128).transpose(2, 1, 0).copy()
    W2 = f32(inputs["pal_W2"])
    b2 = f32(inputs["pal_b2"])
    encw = f32(inputs["enc_W"]).reshape(KT, 128).T.copy()
    selw = f32(inputs["sel_W"]).reshape(T, KT, 128).transpose(2, 1, 0).copy()
    selb = f32(inputs["sel_b"]).reshape(1, T)
    lng = f32(inputs["ln_g"]).reshape(1, H)
    lnb = f32(inputs["ln_b"]).reshape(1, H)
    shared = dict(Wd=Wd, dbias=dbias, W1=W1, b1=b1, W2=W2, b2=b2, encw=encw,
                  selw=selw, selb=selb, lng=lng, lnb=lnb)
    return [
        dict(hs=hs[b], inp=inp[b], mask=msk[b:b + 1], **shared)
        for b in range(B)
    ]


def kernel(**inputs):
    nc = _get_nc(**_flags(inputs))
    res = run_bass_kernel_spmd(nc, _in_maps(inputs), list(range(N_CORES)))
    out = np.stack([res.results[b]["out"] for b in range(B)], axis=0)
    return out


# revision 22
# speedup vs baseline: 1.1364x; 1.1364x over previous
"""Trainium2 Bass kernel for nn_BertSelfOutputPAL.

Data-parallel over batch: 8 batch elements -> 8 NeuronCores, no collectives.
Per core (batch element b), with S=2048, H=1024, P=256, T=4:
  h   = hs @ W + b                       (dense)
  low_t = h @ W1[t] + b1[t]              (PAL down-proj, T branches)
  ts_t  = low_t @ W2[t] + b2[t]          (PAL up-proj)
  tw  = softmax(h @ enc_W + mask)        (token gate over S)
  tv  = tw @ h
  td  = softmax(tv @ sel_W.T + sel_b)    (task gate over T)
  x   = h + input + sum_t td[t] * ts_t
  out = LayerNorm(x) * g + beta

On-chip layout: activations are kept feature-major ("hT": [H partitions, S
free]) so natural-layout weights slice directly as matmul stationary
operands. The residual/LayerNorm stage runs in natural layout [S, H]; h is
transposed back via PE-transpose matmuls accumulated directly into the
stage-2 PSUM tile. Matmuls run as float32r (full-rate fp32 data).
"""

import os
import numpy as np
from contextlib import ExitStack

import concourse.bacc as bacc
import concourse.bass as bass
import concourse.mybir as mybir
import concourse.tile as tile
from concourse.bass_utils import run_bass_kernel_spmd
from concourse.masks import make_identity

FP = mybir.dt.float32
FR = mybir.dt.float32r
AF = mybir.ActivationFunctionType
ALU = mybir.AluOpType
AX = mybir.AxisListType
EPS = 1e-12

B, S_FULL, H, P, T = 8, 2048, 1024, 256, 4
KT = H // 128      # 8 h-tiles
PT = P // 128      # 2 p-tiles
N_CORES = 8


def fr(ap):
    return ap.bitcast(FR)


def build_nc(S=S_FULL, zb2=False, zmask=False, zg=False, zb=False):
    SC = S // 512            # 512-wide s-chunks
    NST = S // 128           # 128-row s-tiles
    nc = bacc.Bacc("TRN2", target_bir_lowering=False, debug=False)

    # ---- DRAM I/O (per-core) ----
    hs = nc.dram_tensor("hs", [S, H], FR, kind="ExternalInput").ap()
    inp = nc.dram_tensor("inp", [S, H], FP, kind="ExternalInput").ap()
    mask = nc.dram_tensor("mask", [1, S], FP, kind="ExternalInput").ap()
    Wd = nc.dram_tensor("Wd", [H, H], FR, kind="ExternalInput").ap()
    dbias_d = nc.dram_tensor("dbias", [128, KT], FP, kind="ExternalInput").ap()
    W1d = nc.dram_tensor("W1", [T, H, P], FR, kind="ExternalInput").ap()
    b1_d = nc.dram_tensor("b1", [128, PT, T], FP, kind="ExternalInput").ap()
    W2d = nc.dram_tensor("W2", [T, P, H], FR, kind="ExternalInput").ap()
    b2_d = nc.dram_tensor("b2", [T, H], FR, kind="ExternalInput").ap()
    encw_d = nc.dram_tensor("encw", [128, KT], FR, kind="ExternalInput").ap()
    selw_d = nc.dram_tensor("selw", [128, KT, T], FR, kind="ExternalInput").ap()
    selb_d = nc.dram_tensor("selb", [1, T], FP, kind="ExternalInput").ap()
    lng_d = nc.dram_tensor("lng", [1, H], FP, kind="ExternalInput").ap()
    lnb_d = nc.dram_tensor("lnb", [1, H], FP, kind="ExternalInput").ap()
    outp = nc.dram_tensor("out", [S, H], FP, kind="ExternalOutput").ap()

    with tile.TileContext(nc) as tc, ExitStack() as ctx:
        # ---------- persistent pools ----------
        persist = ctx.enter_context(tc.tile_pool(name="persist", bufs=1))
        htp = ctx.enter_context(tc.tile_pool(name="htp", bufs=1))

        ident = persist.tile([128, 128], FP, tag="ident")
        make_identity(nc, ident[:])
        ones1 = persist.tile([1, 128], FR, tag="ones1")
        nc.gpsimd.memset(ones1[:], 1.0)

        dbias = persist.tile([128, KT], FP, tag="dbias")
        nc.sync.dma_start(dbias[:], dbias_d)
        b1s = persist.tile([128, PT, T], FP, tag="b1s")
        nc.sync.dma_start(b1s[:], b1_d)
        encw = persist.tile([128, KT], FR, tag="encw")
        nc.sync.dma_start(encw[:], encw_d)
        selw = persist.tile([128, KT, T], FR, tag="selw")
        nc.sync.dma_start(selw[:], selw_d)
        selb = persist.tile([1, T], FP, tag="selb")
        nc.sync.dma_start(selb[:], selb_d)
        b2n = persist.tile([T, H], FP, tag="b2n")
        nc.sync.dma_start(b2n[:], b2_d)
        lngr = persist.tile([1, H], FP, tag="lngr")
        nc.sync.dma_start(lngr[:], lng_d)
        lnbr = persist.tile([1, H], FP, tag="lnbr")
        nc.sync.dma_start(lnbr[:], lnb_d)
        lngb = persist.tile([128, H], FP, tag="lngb")
        nc.gpsimd.partition_broadcast(lngb[:], lngr[:])
        lnbb = persist.tile([128, H], FP, tag="lnbb")
        nc.gpsimd.partition_broadcast(lnbb[:], lnbr[:])

        # logits row; reused in place as exp/tw
        lgrow = persist.tile([1, S], FP, tag="lgrow")
        tvcols = persist.tile([128, KT], FP, tag="tvcols")
        td_row = persist.tile([1, T], FP, tag="td_row")
        tdcol = persist.tile([T, 1], FP, tag="tdcol")
        b2c = persist.tile([1, H], FP, tag="b2c")
        tdb = persist.tile([128, T], FP, tag="tdb")

        # hT: feature-major h, [128, S] x KT tiles (persists all passes)
        hT = [htp.tile([128, S], FP, tag=f"ht{k}") for k in range(KT)]

        # ================= pass A: transpose X, dense, logits =================
        with tc.tile_pool(name="pA", bufs=1) as pa, \
             tc.tile_pool(name="pA_xn", bufs=6) as pxn, \
             tc.tile_pool(name="pA_xt", bufs=2 * KT) as pxt, \
             tc.tile_pool(name="pA_ps_t", bufs=3, space="PSUM") as tps, \
             tc.tile_pool(name="pA_ps_d", bufs=3, space="PSUM") as dps, \
             tc.tile_pool(name="pA_ps_l", bufs=1, space="PSUM") as lps:
            W_sb = [pa.tile([128, H], FR, tag=f"wd{k}") for k in range(KT)]
            for k in range(KT):
                nc.sync.dma_start(W_sb[k][:], Wd[k * 128:(k + 1) * 128, :])
            mrow = pa.tile([1, S], FP, tag="mrow")
            nc.sync.dma_start(mrow[:], mask)

            for c in range(SC):
                # load X natural rows for this chunk
                xn = []
                for st in range(4):
                    t_ = pxn.tile([128, H], FP, tag="xn")
                    nc.sync.dma_start(t_[:], hs[(c * 4 + st) * 128:(c * 4 + st + 1) * 128, :])
                    xn.append(t_)
                # transpose -> XT chunk tiles [128(h), 512(s)]
                xtc = []
                for kt in range(KT):
                    ps = tps.tile([128, 512], FP, tag="tps")
                    for st in range(4):
                        nc.tensor.transpose(
                            fr(ps[:, st * 128:(st + 1) * 128]),
                            xn[st][:, kt * 128:(kt + 1) * 128],
                            identr[:],
                        )
                    xt_t = pxt.tile([128, 512], FP, tag="xt")
                    nc.scalar.copy(fr(xt_t[:]), ps[:])
                    xtc.append(xt_t)
                # dense: hT[mt][:, chunk] = sum_k W[k,mt].T @ XT[k] (+bias)
                for mt in range(KT):
                    ps = dps.tile([128, 512], FP, tag="dps")
                    for kt in range(KT):
                        nc.tensor.matmul(
                            ps[:],
                            fr(W_sb[kt][:, mt * 128:(mt + 1) * 128]),
                            fr(xtc[kt][:]),
                            start=(kt == 0),
                            stop=(kt == KT - 1),
                        )
                    nc.scalar.activation(
                        fr(hT[mt][:, c * 512:(c + 1) * 512]), ps[:], AF.Identity,
                        bias=dbias[:, mt:mt + 1], scale=1.0,
                    )
                # logits for this chunk (uses hT)
                ps = lps.tile([1, 512], FP, tag="lps")
                for kt in range(KT):
                    nc.tensor.matmul(
                        ps[:],
                        fr(encw[:, kt:kt + 1]),
                        fr(hT[kt][:, c * 512:(c + 1) * 512]),
                        start=(kt == 0),
                        stop=(kt == KT - 1),
                    )
                # logits + mask -> lgrow chunk
                if zmask:
                    nc.scalar.copy(lgrow[:, c * 512:(c + 1) * 512], ps[:])
                else:
                    nc.vector.tensor_add(
                        lgrow[:, c * 512:(c + 1) * 512], ps[:],
                        mrow[:, c * 512:(c + 1) * 512],
                    )

        # ================= W1/W2 loads (region freed by pass A) ===============
        w12 = ctx.enter_context(tc.tile_pool(name="w12", bufs=1))
        W1_sb = {}
        for t in range(T):
            for kt in range(KT):
                w1t = w12.tile([128, P], FR, tag=f"w1_{t}_{kt}")
                nc.sync.dma_start(w1t[:], W1d[t, kt * 128:(kt + 1) * 128, :])
                W1_sb[(t, kt)] = w1t
        W2_sb = {}
        for t in range(T):
            for pt in range(PT):
                w2t = w12.tile([128, H], FR, tag=f"w2_{t}_{pt}")
                nc.sync.dma_start(w2t[:], W2d[t, pt * 128:(pt + 1) * 128, :])
                W2_sb[(t, pt)] = w2t

        lowp = ctx.enter_context(tc.tile_pool(name="lowp", bufs=2 * T * PT))
        lowps = ctx.enter_context(
            tc.tile_pool(name="lowps", bufs=3 if zb2 else 2, space="PSUM"))
        xps = ctx.enter_context(tc.tile_pool(name="xps", bufs=4, space="PSUM"))
        bps = ctx.enter_context(
            tc.tile_pool(name="bps", bufs=1 if zb2 else 2, space="PSUM"))

        # ================= pass B: softmax, tv, td, scale W2 ==================
        with tc.tile_pool(name="pB", bufs=1) as pb:
            mx = pb.tile([1, 1], FP, tag="mx")
            nc.vector.reduce_max(mx[:], lgrow[:], axis=AX.X)
            negmx = pb.tile([1, 1], FP, tag="negmx")
            nc.vector.tensor_scalar(negmx[:], mx[:], -1.0, None, op0=ALU.mult)
            zsum = pb.tile([1, 1], FP, tag="zsum")
            # lgrow <- exp(lgrow - mx), zsum = sum(exp)
            nc.scalar.activation(lgrow[:], lgrow[:], AF.Exp,
                                 bias=negmx[:], scale=1.0, accum_out=zsum[:])
            rz = pb.tile([1, 1], FP, tag="rz")
            nc.vector.reciprocal(rz[:], zsum[:])
            # lgrow <- tw = exp * (1/Z)
            nc.vector.tensor_scalar(lgrow[:], lgrow[:], rz[:], None, op0=ALU.mult)
            twb = pb.tile([128, S], FP, tag="twb")
            nc.gpsimd.partition_broadcast(twb[:], lgrow[:])
            # tv[h] = sum_s hT[h,s]*tw[s]  (DVE fused mul+reduce, halves)
            scr = pb.tile([128, S // 2], FP, tag="scr")
            tva = pb.tile([128, KT], FP, tag="tva")
            tvb_ = pb.tile([128, KT], FP, tag="tvb")
            HF = S // 2
            for kt in range(KT):
                nc.vector.scalar_tensor_tensor(
                    scr[:], hT[kt][:, :HF], 1.0, twb[:, :HF],
                    op0=ALU.mult, op1=ALU.mult, accum_out=tva[:, kt:kt + 1])
                nc.vector.scalar_tensor_tensor(
                    scr[:], hT[kt][:, HF:], 1.0, twb[:, HF:],
                    op0=ALU.mult, op1=ALU.mult, accum_out=tvb_[:, kt:kt + 1])
            nc.vector.tensor_add(fr(tvcols[:]), tva[:], tvb_[:])
            # td logits = tv @ selW.T + selb  -> [1, T]
            ps = bps.tile([1, T], FP, tag="bmisc")
            for kt in range(KT):
                nc.tensor.matmul(ps[:], fr(tvcols[:, kt:kt + 1]), fr(selw[:, kt, :]),
                                 start=(kt == 0), stop=(kt == KT - 1))
            tdl = pb.tile([1, T], FP, tag="tdl")
            nc.vector.tensor_add(tdl[:], ps[:], selb[:])
            # softmax over T
            mx2 = pb.tile([1, 1], FP, tag="mx2")
            nc.vector.reduce_max(mx2[:], tdl[:], axis=AX.X)
            negmx2 = pb.tile([1, 1], FP, tag="negmx2")
            nc.vector.tensor_scalar(negmx2[:], mx2[:], -1.0, None, op0=ALU.mult)
            z2 = pb.tile([1, 1], FP, tag="z2")
            nc.scalar.activation(tdl[:], tdl[:], AF.Exp, bias=negmx2[:], scale=1.0,
                                 accum_out=z2[:])
            rz2 = pb.tile([1, 1], FP, tag="rz2")
            nc.vector.reciprocal(rz2[:], z2[:])
            nc.vector.tensor_scalar(fr(td_row[:]), tdl[:], rz2[:], None, op0=ALU.mult)
            # td as column [T,1] via K=1 matmul: td_row.T @ [[1]]
            ps2 = bps.tile([T, 1], FP, tag="bmisc")
            nc.tensor.matmul(ps2[:], fr(td_row[:]), fr(ones1[:, :1]),
                             start=True, stop=True)
            nc.scalar.copy(fr(tdcol[:]), ps2[:])
            # b2c row = td @ b2  [1, H]
            for hc in range(2):
                ps3 = bps.tile([1, 512], FP, tag="bmisc")
                nc.tensor.matmul(ps3[:], fr(tdcol[:]),
                                 fr(b2n[:, hc * 512:(hc + 1) * 512]),
                                 start=True, stop=True)
                nc.scalar.copy(fr(b2c[:, hc * 512:(hc + 1) * 512]), ps3[:])
            # td broadcast [128, T]; scale W2 in place
            nc.gpsimd.partition_broadcast(tdb[:], td_row[:])
            for t in range(T):
                for pt in range(PT):
                    w2t = W2_sb[(t, pt)]
                    nc.vector.tensor_scalar(fr(w2t[:]), w2t[:], tdb[:, t:t + 1], None,
                                            op0=ALU.mult)

        # ================= pass C: low, stage2(+transpose+bias), LN ===========
        st3 = ctx.enter_context(tc.tile_pool(name="st3", bufs=1))
        xt_pool = ctx.enter_context(tc.tile_pool(name="xt3", bufs=2))
        in_pool = ctx.enter_context(tc.tile_pool(name="in3", bufs=2))
        stats = ctx.enter_context(tc.tile_pool(name="stats", bufs=4))

        low_tiles = {}

        def emit_low(c):
            for t in range(T):
                for pt in range(PT):
                    ps = lowps.tile([128, 512], FP, tag="lowps")
                    for kt in range(KT):
                        nc.tensor.matmul(
                            ps[:],
                            fr(W1_sb[(t, kt)][:, pt * 128:(pt + 1) * 128]),
                            fr(hT[kt][:, c * 512:(c + 1) * 512]),
                            start=(kt == 0), stop=(kt == KT - 1),
                        )
                    lt = lowp.tile([128, 512], FP, tag="low")
                    nc.scalar.activation(fr(lt[:]), ps[:], AF.Identity,
                                         bias=b1s[:, pt:pt + 1, t:t + 1], scale=1.0)
                    low_tiles[(c, t, pt)] = lt

        def emit_stage23(c):
            for st in range(4):
                s_abs = c * 4 + st
                pss = []
                for hc in range(2):
                    ps = xps.tile([128, 512], FP, tag="xps")
                    k = 0
                    for t in range(T):
                        for pt in range(PT):
                            nc.tensor.matmul(
                                ps[:],
                                fr(low_tiles[(c, t, pt)][:, st * 128:(st + 1) * 128]),
                                fr(W2_sb[(t, pt)][:, hc * 512:(hc + 1) * 512]),
                                start=(k == 0), stop=False,
                            )
                            k += 1
                    # accumulate h_nat via PE transpose of hT into same psum
                    for j in range(4):
                        kt = hc * 4 + j
                        nc.tensor.matmul(
                            fr(ps[:, j * 128:(j + 1) * 128]),
                            fr(hT[kt][:, s_abs * 128:(s_abs + 1) * 128]),
                            identr[:],
                            is_transpose=True, start=False, stop=(zb2 and j == 3),
                        )
                    if not zb2:
                        # + b2c broadcast row (K=1 rank-1 update), ends group
                        nc.tensor.matmul(
                            ps[:], ones1[:], fr(b2c[:, hc * 512:(hc + 1) * 512]),
                            start=False, stop=True,
                        )
                    pss.append(ps)
                # ---- stage 3: x = psum + input; LayerNorm ----
                it = in_pool.tile([128, H], FP, tag="inp")
                nc.sync.dma_start(it[:], inp[s_abs * 128:(s_abs + 1) * 128, :])
                xt_ = xt_pool.tile([128, H], FP, tag="x")
                s0 = stats.tile([128, 1], FP, tag="s0")
                s1 = stats.tile([128, 1], FP, tag="s1")
                for hc, sacc in ((0, s0), (1, s1)):
                    nc.vector.scalar_tensor_tensor(
                        xt_[:, hc * 512:(hc + 1) * 512], pss[hc][:], 0.0,
                        it[:, hc * 512:(hc + 1) * 512],
                        op0=ALU.add, op1=ALU.add, accum_out=sacc[:])
                sq = sq_pool.tile([128, H], FP, tag="sq")
                ssq = stats.tile([128, 1], FP, tag="ssq")
                nc.scalar.activation(sq[:], xt_[:], AF.Square, bias=zerot[:], accum_out=ssq[:])
                ssum = stats.tile([128, 1], FP, tag="ssum")
                nc.vector.tensor_add(ssum[:], s0[:], s1[:])
                mu = stats.tile([128, 1], FP, tag="mu")
                nc.vector.tensor_scalar(mu[:], ssum[:], 1.0 / H, None, op0=ALU.mult)
                musq = stats.tile([128, 1], FP, tag="musq")
                nc.vector.tensor_mul(musq[:], mu[:], mu[:])
                var = stats.tile([128, 1], FP, tag="var")
                nc.vector.tensor_scalar(var[:], ssq[:], 1.0 / H, musq[:],
                                        op0=ALU.mult, op1=ALU.subtract)
                sd = stats.tile([128, 1], FP, tag="sd")
                nc.scalar.activation(sd[:], var[:], AF.Sqrt, bias=epst[:], scale=1.0)
                isd = stats.tile([128, 1], FP, tag="isd")
                nc.vector.reciprocal(isd[:], sd[:])
                # x <- (x - mu) * isd   (2x-mode tensor_scalar)
                nc.vector.tensor_scalar(xt_[:], xt_[:], mu[:], isd[:],
                                        op0=ALU.subtract, op1=ALU.mult)
                if not zg:
                    # x <- x * g
                    nc.vector.scalar_tensor_tensor(xt_[:], xt_[:], 1.0, lngb[:],
                                                   op0=ALU.mult, op1=ALU.mult)
                if not zb:
                    # x <- x + beta (gpsimd), then store
                    nc.gpsimd.tensor_add(xt_[:], xt_[:], lnbb[:])
                nc.sync.dma_start(outp[s_abs * 128:(s_abs + 1) * 128, :], xt_[:])

        # software pipeline: keep 2 chunks of low tiles in flight
        emit_low(0)
        for c in range(1, SC):
            emit_low(c)
            emit_stage23(c - 1)
        emit_stage23(SC - 1)

    nc.finalize()
    return nc


_CACHE = {}


def _get_nc(S=S_FULL, zb2=False, zmask=False, zg=False, zb=False):
    key = (S, zb2, zmask, zg, zb)
    if key not in _CACHE:
        _CACHE[key] = build_nc(S, zb2=zb2, zmask=zmask, zg=zg, zb=zb)
    return _CACHE[key]


def _flags(inputs):
    f32 = lambda x: np.asarray(x, dtype=np.float32)
    return dict(
        zb2=not np.any(f32(inputs["pal_b2"])),
        zmask=not np.any(f32(inputs["attention_mask"])),
        zg=bool(np.all(f32(inputs["ln_g"]) == 1.0)),
        zb=not np.any(f32(inputs["ln_b"])),
    )


def _in_maps(inputs, S=S_FULL):
    f32 = lambda x: np.ascontiguousarray(np.asarray(x), dtype=np.float32)
    hs = f32(inputs["hidden_states"])
    inp = f32(inputs["input_tensor"])
    msk = f32(inputs["attention_mask"]).reshape(B, S)
    Wd = f32(inputs["dense_W"])
    dbias = f32(inputs["dense_b"]).reshape(KT, 128).T.copy()
    W1 = f32(inputs["pal_W1"])
    b1 = f32(inputs["pal_b1"]).reshape(T, PT, 128).transpose(2, 1, 0).copy()
    W2 = f32(inputs["pal_W2"])
    b2 = f32(inputs["pal_b2"])
    encw = f32(inputs["enc_W"]).reshape(KT, 128).T.copy()
    selw = f32(inputs["sel_W"]).reshape(T, KT, 128).transpose(2, 1, 0).copy()
    selb = f32(inputs["sel_b"]).reshape(1, T)
    lng = f32(inputs["ln_g"]).reshape(1, H)
    lnb = f32(inputs["ln_b"]).reshape(1, H)
    shared = dict(Wd=Wd, dbias=dbias, W1=W1, b1=b1, W2=W2, b2=b2, encw=encw,
                  selw=selw, selb=selb, lng=lng, lnb=lnb)
    return [
        dict(hs=hs[b], inp=inp[b], mask=msk[b:b + 1], **shared)
        for b in range(B)
    ]


def kernel(**inputs):
    nc = _get_nc(**_flags(inputs))
    res = run_bass_kernel_spmd(nc, _in_maps(inputs), list(range(N_CORES)))
    out = np.stack([res.results[b]["out"] for b in range(B)], axis=0)
    return out


# revision 23
# speedup vs baseline: 1.1446x; 1.0072x over previous
"""Trainium2 Bass kernel for nn_BertSelfOutputPAL.

Data-parallel over batch: 8 batch elements -> 8 NeuronCores, no collectives.
Per core (batch element b), with S=2048, H=1024, P=256, T=4:
  h   = hs @ W + b                       (dense)
  low_t = h @ W1[t] + b1[t]              (PAL down-proj, T branches)
  ts_t  = low_t @ W2[t] + b2[t]          (PAL up-proj)
  tw  = softmax(h @ enc_W + mask)        (token gate over S)
  tv  = tw @ h
  td  = softmax(tv @ sel_W.T + sel_b)    (task gate over T)
  x   = h + input + sum_t td[t] * ts_t
  out = LayerNorm(x) * g + beta

On-chip layout: activations are kept feature-major ("hT": [H partitions, S
free]) so natural-layout weights slice directly as matmul stationary
operands. The residual/LayerNorm stage runs in natural layout [S, H]; h is
transposed back via PE-transpose matmuls accumulated directly into the
stage-2 PSUM tile. Matmuls run as float32r (full-rate fp32 data).
"""

import os
import numpy as np
from contextlib import ExitStack

import concourse.bacc as bacc
import concourse.bass as bass
import concourse.mybir as mybir
import concourse.tile as tile
from concourse.bass_utils import run_bass_kernel_spmd
from concourse.masks import make_identity

FP = mybir.dt.float32
FR = mybir.dt.float32r
AF = mybir.ActivationFunctionType
ALU = mybir.AluOpType
AX = mybir.AxisListType
EPS = 1e-12

B, S_FULL, H, P, T = 8, 2048, 1024, 256, 4
KT = H // 128      # 8 h-tiles
PT = P // 128      # 2 p-tiles
N_CORES = 8


def fr(ap):
    return ap.bitcast(FR)


def build_nc(S=S_FULL, zb2=False, zmask=False, zg=False, zb=False):
    SC = S // 512            # 512-wide s-chunks
    NST = S // 128           # 128-row s-tiles
    nc = bacc.Bacc("TRN2", target_bir_lowering=False, debug=False)

    # ---- DRAM I/O (per-core) ----
    hs = nc.dram_tensor("hs", [S, H], FR, kind="ExternalInput").ap()
    inp = nc.dram_tensor("inp", [S, H], FP, kind="ExternalInput").ap()
    mask = nc.dram_tensor("mask", [1, S], FP, kind="ExternalInput").ap()
    Wd = nc.dram_tensor("Wd", [H, H], FR, kind="ExternalInput").ap()
    dbias_d = nc.dram_tensor("dbias", [128, KT], FP, kind="ExternalInput").ap()
    W1d = nc.dram_tensor("W1", [T, H, P], FR, kind="ExternalInput").ap()
    b1_d = nc.dram_tensor("b1", [128, PT, T], FP, kind="ExternalInput").ap()
    W2d = nc.dram_tensor("W2", [T, P, H], FR, kind="ExternalInput").ap()
    b2_d = nc.dram_tensor("b2", [T, H], FR, kind="ExternalInput").ap()
    encw_d = nc.dram_tensor("encw", [128, KT], FR, kind="ExternalInput").ap()
    selw_d = nc.dram_tensor("selw", [128, KT, T], FR, kind="ExternalInput").ap()
    selb_d = nc.dram_tensor("selb", [1, T], FP, kind="ExternalInput").ap()
    lng_d = nc.dram_tensor("lng", [1, H], FP, kind="ExternalInput").ap()
    lnb_d = nc.dram_tensor("lnb", [1, H], FP, kind="ExternalInput").ap()
    outp = nc.dram_tensor("out", [S, H], FP, kind="ExternalOutput").ap()

    with tile.TileContext(nc) as tc, ExitStack() as ctx:
        # ---------- persistent pools ----------
        persist = ctx.enter_context(tc.tile_pool(name="persist", bufs=1))
        htp = ctx.enter_context(tc.tile_pool(name="htp", bufs=1))

        ident = persist.tile([128, 128], FP, tag="ident")
        make_identity(nc, ident[:])
        ones1 = persist.tile([1, 128], FR, tag="ones1")
        nc.gpsimd.memset(ones1[:], 1.0)

        dbias = persist.tile([128, KT], FP, tag="dbias")
        nc.sync.dma_start(dbias[:], dbias_d)
        b1s = persist.tile([128, PT, T], FP, tag="b1s")
        nc.sync.dma_start(b1s[:], b1_d)
        encw = persist.tile([128, KT], FR, tag="encw")
        nc.sync.dma_start(encw[:], encw_d)
        selw = persist.tile([128, KT, T], FR, tag="selw")
        nc.sync.dma_start(selw[:], selw_d)
        selb = persist.tile([1, T], FP, tag="selb")
        nc.sync.dma_start(selb[:], selb_d)
        b2n = persist.tile([T, H], FP, tag="b2n")
        nc.sync.dma_start(b2n[:], b2_d)
        lngr = persist.tile([1, H], FP, tag="lngr")
        nc.sync.dma_start(lngr[:], lng_d)
        lnbr = persist.tile([1, H], FP, tag="lnbr")
        nc.sync.dma_start(lnbr[:], lnb_d)
        lngb = persist.tile([128, H], FP, tag="lngb")
        nc.gpsimd.partition_broadcast(lngb[:], lngr[:])
        lnbb = persist.tile([128, H], FP, tag="lnbb")
        nc.gpsimd.partition_broadcast(lnbb[:], lnbr[:])

        # logits row; reused in place as exp/tw
        lgrow = persist.tile([1, S], FP, tag="lgrow")
        tvcols = persist.tile([128, KT], FP, tag="tvcols")
        td_row = persist.tile([1, T], FP, tag="td_row")
        tdcol = persist.tile([T, 1], FP, tag="tdcol")
        b2c = persist.tile([1, H], FP, tag="b2c")
        tdb = persist.tile([128, T], FP, tag="tdb")

        # hT: feature-major h, [128, S] x KT tiles (persists all passes)
        hT = [htp.tile([128, S], FP, tag=f"ht{k}") for k in range(KT)]

        # ================= pass A: transpose X, dense, logits =================
        with tc.tile_pool(name="pA", bufs=1) as pa, \
             tc.tile_pool(name="pA_xn", bufs=8) as pxn, \
             tc.tile_pool(name="pA_xt", bufs=2 * KT) as pxt, \
             tc.tile_pool(name="pA_ps_t", bufs=3, space="PSUM") as tps, \
             tc.tile_pool(name="pA_ps_d", bufs=3, space="PSUM") as dps, \
             tc.tile_pool(name="pA_ps_l", bufs=1, space="PSUM") as lps:
            W_sb = [pa.tile([128, H], FR, tag=f"wd{k}") for k in range(KT)]
            for k in range(KT):
                nc.sync.dma_start(W_sb[k][:], Wd[k * 128:(k + 1) * 128, :])
            mrow = pa.tile([1, S], FP, tag="mrow")
            nc.sync.dma_start(mrow[:], mask)

            for c in range(SC):
                # load X natural rows for this chunk
                xn = []
                for st in range(4):
                    t_ = pxn.tile([128, H], FP, tag="xn")
                    nc.sync.dma_start(t_[:], hs[(c * 4 + st) * 128:(c * 4 + st + 1) * 128, :])
                    xn.append(t_)
                # transpose -> XT chunk tiles [128(h), 512(s)]
                xtc = []
                for kt in range(KT):
                    ps = tps.tile([128, 512], FP, tag="tps")
                    for st in range(4):
                        nc.tensor.transpose(
                            fr(ps[:, st * 128:(st + 1) * 128]),
                            xn[st][:, kt * 128:(kt + 1) * 128],
                            identr[:],
                        )
                    xt_t = pxt.tile([128, 512], FP, tag="xt")
                    nc.vector.tensor_copy(fr(xt_t[:]), ps[:])
                    xtc.append(xt_t)
                # dense: hT[mt][:, chunk] = sum_k W[k,mt].T @ XT[k] (+bias)
                for mt in range(KT):
                    ps = dps.tile([128, 512], FP, tag="dps")
                    for kt in range(KT):
                        nc.tensor.matmul(
                            ps[:],
                            fr(W_sb[kt][:, mt * 128:(mt + 1) * 128]),
                            fr(xtc[kt][:]),
                            start=(kt == 0),
                            stop=(kt == KT - 1),
                        )
                    nc.scalar.activation(
                        fr(hT[mt][:, c * 512:(c + 1) * 512]), ps[:], AF.Identity,
                        bias=dbias[:, mt:mt + 1], scale=1.0,
                    )
                # logits for this chunk (uses hT)
                ps = lps.tile([1, 512], FP, tag="lps")
                for kt in range(KT):
                    nc.tensor.matmul(
                        ps[:],
                        fr(encw[:, kt:kt + 1]),
                        fr(hT[kt][:, c * 512:(c + 1) * 512]),
                        start=(kt == 0),
                        stop=(kt == KT - 1),
                    )
                # logits + mask -> lgrow chunk
                if zmask:
                    nc.scalar.copy(lgrow[:, c * 512:(c + 1) * 512], ps[:])
                else:
                    nc.vector.tensor_add(
                        lgrow[:, c * 512:(c + 1) * 512], ps[:],
                        mrow[:, c * 512:(c + 1) * 512],
                    )

        # ================= W1/W2 loads (region freed by pass A) ===============
        w12 = ctx.enter_context(tc.tile_pool(name="w12", bufs=1))
        W1_sb = {}
        for t in range(T):
            for kt in range(KT):
                w1t = w12.tile([128, P], FR, tag=f"w1_{t}_{kt}")
                nc.sync.dma_start(w1t[:], W1d[t, kt * 128:(kt + 1) * 128, :])
                W1_sb[(t, kt)] = w1t
        W2_sb = {}
        for t in range(T):
            for pt in range(PT):
                w2t = w12.tile([128, H], FR, tag=f"w2_{t}_{pt}")
                nc.sync.dma_start(w2t[:], W2d[t, pt * 128:(pt + 1) * 128, :])
                W2_sb[(t, pt)] = w2t

        lowp = ctx.enter_context(tc.tile_pool(name="lowp", bufs=2 * T * PT))
        lowps = ctx.enter_context(
            tc.tile_pool(name="lowps", bufs=3 if zb2 else 2, space="PSUM"))
        xps = ctx.enter_context(tc.tile_pool(name="xps", bufs=4, space="PSUM"))
        bps = ctx.enter_context(
            tc.tile_pool(name="bps", bufs=1 if zb2 else 2, space="PSUM"))

        # ================= pass B: softmax, tv, td, scale W2 ==================
        with tc.tile_pool(name="pB", bufs=1) as pb:
            mx = pb.tile([1, 1], FP, tag="mx")
            nc.vector.reduce_max(mx[:], lgrow[:], axis=AX.X)
            negmx = pb.tile([1, 1], FP, tag="negmx")
            nc.vector.tensor_scalar(negmx[:], mx[:], -1.0, None, op0=ALU.mult)
            zsum = pb.tile([1, 1], FP, tag="zsum")
            # lgrow <- exp(lgrow - mx), zsum = sum(exp)
            nc.scalar.activation(lgrow[:], lgrow[:], AF.Exp,
                                 bias=negmx[:], scale=1.0, accum_out=zsum[:])
            rz = pb.tile([1, 1], FP, tag="rz")
            nc.vector.reciprocal(rz[:], zsum[:])
            # lgrow <- tw = exp * (1/Z)
            nc.vector.tensor_scalar(lgrow[:], lgrow[:], rz[:], None, op0=ALU.mult)
            twb = pb.tile([128, S], FP, tag="twb")
            nc.gpsimd.partition_broadcast(twb[:], lgrow[:])
            # tv[h] = sum_s hT[h,s]*tw[s]  (DVE fused mul+reduce, halves)
            scr = pb.tile([128, S // 2], FP, tag="scr")
            tva = pb.tile([128, KT], FP, tag="tva")
            tvb_ = pb.tile([128, KT], FP, tag="tvb")
            HF = S // 2
            for kt in range(KT):
                nc.vector.scalar_tensor_tensor(
                    scr[:], hT[kt][:, :HF], 1.0, twb[:, :HF],
                    op0=ALU.mult, op1=ALU.mult, accum_out=tva[:, kt:kt + 1])
                nc.vector.scalar_tensor_tensor(
                    scr[:], hT[kt][:, HF:], 1.0, twb[:, HF:],
                    op0=ALU.mult, op1=ALU.mult, accum_out=tvb_[:, kt:kt + 1])
            nc.vector.tensor_add(fr(tvcols[:]), tva[:], tvb_[:])
            # td logits = tv @ selW.T + selb  -> [1, T]
            ps = bps.tile([1, T], FP, tag="bmisc")
            for kt in range(KT):
                nc.tensor.matmul(ps[:], fr(tvcols[:, kt:kt + 1]), fr(selw[:, kt, :]),
                                 start=(kt == 0), stop=(kt == KT - 1))
            tdl = pb.tile([1, T], FP, tag="tdl")
            nc.vector.tensor_add(tdl[:], ps[:], selb[:])
            # softmax over T
            mx2 = pb.tile([1, 1], FP, tag="mx2")
            nc.vector.reduce_max(mx2[:], tdl[:], axis=AX.X)
            negmx2 = pb.tile([1, 1], FP, tag="negmx2")
            nc.vector.tensor_scalar(negmx2[:], mx2[:], -1.0, None, op0=ALU.mult)
            z2 = pb.tile([1, 1], FP, tag="z2")
            nc.scalar.activation(tdl[:], tdl[:], AF.Exp, bias=negmx2[:], scale=1.0,
                                 accum_out=z2[:])
            rz2 = pb.tile([1, 1], FP, tag="rz2")
            nc.vector.reciprocal(rz2[:], z2[:])
            nc.vector.tensor_scalar(fr(td_row[:]), tdl[:], rz2[:], None, op0=ALU.mult)
            # td as column [T,1] via K=1 matmul: td_row.T @ [[1]]
            ps2 = bps.tile([T, 1], FP, tag="bmisc")
            nc.tensor.matmul(ps2[:], fr(td_row[:]), fr(ones1[:, :1]),
                             start=True, stop=True)
            nc.scalar.copy(fr(tdcol[:]), ps2[:])
            # b2c row = td @ b2  [1, H]
            for hc in range(2):
                ps3 = bps.tile([1, 512], FP, tag="bmisc")
                nc.tensor.matmul(ps3[:], fr(tdcol[:]),
                                 fr(b2n[:, hc * 512:(hc + 1) * 512]),
                                 start=True, stop=True)
                nc.scalar.copy(fr(b2c[:, hc * 512:(hc + 1) * 512]), ps3[:])
            # td broadcast [128, T]; scale W2 in place
            nc.gpsimd.partition_broadcast(tdb[:], td_row[:])
            for t in range(T):
                for pt in range(PT):
                    w2t = W2_sb[(t, pt)]
                    nc.vector.tensor_scalar(fr(w2t[:]), w2t[:], tdb[:, t:t + 1], None,
                                            op0=ALU.mult)

        # ================= pass C: low, stage2(+transpose+bias), LN ===========
        st3 = ctx.enter_context(tc.tile_pool(name="st3", bufs=1))
        xt_pool = ctx.enter_context(tc.tile_pool(name="xt3", bufs=2))
        in_pool = ctx.enter_context(tc.tile_pool(name="in3", bufs=2))
        stats = ctx.enter_context(tc.tile_pool(name="stats", bufs=4))

        low_tiles = {}

        def emit_low(c):
            for t in range(T):
                for pt in range(PT):
                    ps = lowps.tile([128, 512], FP, tag="lowps")
                    for kt in range(KT):
                        nc.tensor.matmul(
                            ps[:],
                            fr(W1_sb[(t, kt)][:, pt * 128:(pt + 1) * 128]),
                            fr(hT[kt][:, c * 512:(c + 1) * 512]),
                            start=(kt == 0), stop=(kt == KT - 1),
                        )
                    lt = lowp.tile([128, 512], FP, tag="low")
                    nc.scalar.activation(fr(lt[:]), ps[:], AF.Identity,
                                         bias=b1s[:, pt:pt + 1, t:t + 1], scale=1.0)
                    low_tiles[(c, t, pt)] = lt

        def emit_stage23(c):
            for st in range(4):
                s_abs = c * 4 + st
                pss = []
                for hc in range(2):
                    ps = xps.tile([128, 512], FP, tag="xps")
                    k = 0
                    for t in range(T):
                        for pt in range(PT):
                            nc.tensor.matmul(
                                ps[:],
                                fr(low_tiles[(c, t, pt)][:, st * 128:(st + 1) * 128]),
                                fr(W2_sb[(t, pt)][:, hc * 512:(hc + 1) * 512]),
                                start=(k == 0), stop=False,
                            )
                            k += 1
                    # accumulate h_nat via PE transpose of hT into same psum
                    for j in range(4):
                        kt = hc * 4 + j
                        nc.tensor.matmul(
                            fr(ps[:, j * 128:(j + 1) * 128]),
                            fr(hT[kt][:, s_abs * 128:(s_abs + 1) * 128]),
                            identr[:],
                            is_transpose=True, start=False, stop=(zb2 and j == 3),
                        )
                    if not zb2:
                        # + b2c broadcast row (K=1 rank-1 update), ends group
                        nc.tensor.matmul(
                            ps[:], ones1[:], fr(b2c[:, hc * 512:(hc + 1) * 512]),
                            start=False, stop=True,
                        )
                    pss.append(ps)
                # ---- stage 3: x = psum + input; LayerNorm ----
                it = in_pool.tile([128, H], FP, tag="inp")
                nc.sync.dma_start(it[:], inp[s_abs * 128:(s_abs + 1) * 128, :])
                xt_ = xt_pool.tile([128, H], FP, tag="x")
                s0 = stats.tile([128, 1], FP, tag="s0")
                s1 = stats.tile([128, 1], FP, tag="s1")
                for hc, sacc in ((0, s0), (1, s1)):
                    nc.vector.scalar_tensor_tensor(
                        xt_[:, hc * 512:(hc + 1) * 512], pss[hc][:], 0.0,
                        it[:, hc * 512:(hc + 1) * 512],
                        op0=ALU.add, op1=ALU.add, accum_out=sacc[:])
                sq = sq_pool.tile([128, H], FP, tag="sq")
                ssq = stats.tile([128, 1], FP, tag="ssq")
                nc.scalar.activation(sq[:], xt_[:], AF.Square, bias=zerot[:], accum_out=ssq[:])
                ssum = stats.tile([128, 1], FP, tag="ssum")
                nc.vector.tensor_add(ssum[:], s0[:], s1[:])
                mu = stats.tile([128, 1], FP, tag="mu")
                nc.vector.tensor_scalar(mu[:], ssum[:], 1.0 / H, None, op0=ALU.mult)
                musq = stats.tile([128, 1], FP, tag="musq")
                nc.vector.tensor_mul(musq[:], mu[:], mu[:])
                var = stats.tile([128, 1], FP, tag="var")
                nc.vector.tensor_scalar(var[:], ssq[:], 1.0 / H, musq[:],
                                        op0=ALU.mult, op1=ALU.subtract)
                sd = stats.tile([128, 1], FP, tag="sd")
                nc.scalar.activation(sd[:], var[:], AF.Sqrt, bias=epst[:], scale=1.0)
                isd = stats.tile([128, 1], FP, tag="isd")
                nc.vector.reciprocal(isd[:], sd[:])
                # x <- (x - mu) * isd   (2x-mode tensor_scalar)
                nc.vector.tensor_scalar(xt_[:], xt_[:], mu[:], isd[:],
                                        op0=ALU.subtract, op1=ALU.mult)
                if not zg:
                    # x <- x * g
                    nc.vector.scalar_tensor_tensor(xt_[:], xt_[:], 1.0, lngb[:],
                                                   op0=ALU.mult, op1=ALU.mult)
                if not zb:
                    # x <- x + beta (gpsimd), then store
                    nc.gpsimd.tensor_add(xt_[:], xt_[:], lnbb[:])
                nc.sync.dma_start(outp[s_abs * 128:(s_abs + 1) * 128, :], xt_[:])

        # software pipeline: keep 2 chunks of low tiles in flight
        emit_low(0)
        for c in range(1, SC):
            emit_low(c)
            emit_stage23(c - 1)
        emit_stage23(SC - 1)

    nc.finalize()
    return nc


_CACHE = {}


def _get_nc(S=S_FULL, zb2=False, zmask=False, zg=False, zb=False):
    key = (S, zb2, zmask, zg, zb)
    if key not in _CACHE:
        _CACHE[key] = build_nc(S, zb2=zb2, zmask=zmask, zg=zg, zb=zb)
    return _CACHE[key]


def _flags(inputs):
    f32 = lambda x: np.asarray(x, dtype=np.float32)
    return dict(
        zb2=not np.any(f32(inputs["pal_b2"])),
        zmask=not np.any(f32(inputs["attention_mask"])),
        zg=bool(np.all(f32(inputs["ln_g"]) == 1.0)),
        zb=not np.any(f32(inputs["ln_b"])),
    )


def _in_maps(inputs, S=S_FULL):
    f32 = lambda x: np.ascontiguousarray(np.asarray(x), dtype=np.float32)
    hs = f32(inputs["hidden_states"])
    inp = f32(inputs["input_tensor"])
    msk = f32(inputs["attention_mask"]).reshape(B, S)
    Wd = f32(inputs["dense_W"])
    dbias = f32(inputs["dense_b"]).reshape(KT, 128).T.copy()
    W1 = f32(inputs["pal_W1"])
    b1 = f32(inputs["pal_b1"]).reshape(T, PT, 128).transpose(2, 1, 0).copy()
    W2 = f32(inputs["pal_W2"])
    b2 = f32(inputs["pal_b2"])
    encw = f32(inputs["enc_W"]).reshape(KT, 128).T.copy()
    selw = f32(inputs["sel_W"]).reshape(T, KT, 128).transpose(2, 1, 0).copy()
    selb = f32(inputs["sel_b"]).reshape(1, T)
    lng = f32(inputs["ln_g"]).reshape(1, H)
    lnb = f32(inputs["ln_b"]).reshape(1, H)
    shared = dict(Wd=Wd, dbias=dbias, W1=W1, b1=b1, W2=W2, b2=b2, encw=encw,
                  selw=selw, selb=selb, lng=lng, lnb=lnb)
    return [
        dict(hs=hs[b], inp=inp[b], mask=msk[b:b + 1], **shared)
        for b in range(B)
    ]


def kernel(**inputs):
    nc = _get_nc(**_flags(inputs))
    res = run_bass_kernel_spmd(nc, _in_maps(inputs), list(range(N_CORES)))
    out = np.stack([res.results[b]["out"] for b in range(B)], axis=0)
    return out
